# revision 1
# baseline (speedup 1.0000x reference)
"""ExpertLinear (dense MoE blend) Trainium2 kernel.

y[b,o] = sum_k ew[b,k] * (x[b,:] @ W[k,o,:]) + sum_k ew[b,k] * bias[k,o]

Data-parallel over B across 8 cores; each core streams the whole blended
weight tensor. Layout/precision choices:
  - Host pre-transposes W -> wT[k, i, o] (contraction dim i on partitions,
    fully contiguous per-partition DMA rows) and casts it to bf16, with 32
    zero columns appended per row block. bf16 halves the dominant HBM
    stream (32MB -> ~17MB per core) and - critically - lets all 16 weight
    tiles stay live in SBUF at once: no buffer reuse means no DMA needs
    both a WAW and WAR wait, which matters because this walrus build
    accepts at most ONE sync wait per instruction.
  - All small operands (xT i-tiles, ew columns replicated across
    partitions, ewT, bias) are packed host-side into one fp32 tensor `xe`
    and arrive via a single DMA (single semaphore lane).
  - VectorE pre-scales xs_k[i,b] = x[b,i] * ew[b,k] into bf16; the PE then
    accumulates the fp32 bias matmul (ewT.T @ bias, K=8) plus all 128
    bf16 W matmuls into 2 PSUM banks, evicted once at the end.
  - Per W tile, a zero-result matmul (wt-slice.T @ zero-column) absorbs
    the tile's DMA-lane wait on its own PE instruction, so the real
    matmuls carry at most their single DVE-tick wait.
Per-core HBM traffic ~= 18.5 MB; measured numerics ~2e-3 relative L2.
"""

import numpy as np

B, E, IN, OUT = 512, 8, 1024, 1024
NCORES = 8
BL = B // NCORES  # 64 rows per core
P = 128
NIT = IN // P  # 8 i-tiles
W_DMA_ITILES = 16  # i-tiles per W DMA
OUTP = OUT + 32  # zero-padded row length in the bf16 W stream
NTILES = (E * NIT) // W_DMA_ITILES  # 16 weight tiles, all live in SBUF

# xe column layout (float32, 128 partitions)
XT_C = 0                       # 8 i-tiles of xT: [128, 8*64]
EWB_C = XT_C + NIT * BL        # ew columns replicated: [128, 8*64]
EWT_C = EWB_C + E * BL         # ewT on partitions 0..7: [8, 64]
BIAS_C = EWT_C + BL            # bias on partitions 0..7: [8, 1024]
XE_COLS = BIAS_C + OUT

_compiled = None


def _patch_drain_split():
    """The walrus build in this container rejects any instruction carrying
    more than one sync wait, including the kernel-tail Drain that
    TileContext emits with one wait per active semaphore. Split it into a
    sequence of single-wait drains (sequencer-FIFO keeps them ordered;
    the set of waits is identical)."""
    import concourse.tile as tile_mod

    if getattr(tile_mod.TileContext, "_drain_split_patched", False):
        return
    from concourse.tile_sem_assignment import N_PROCS
    from concourse.vector_clock import ScopedClock, VectorClock

    def _drain_and_barrier(self, tick_clock, wait_clock):
        gc = tick_clock.global_clock
        for p in range(N_PROCS):
            t = gc[p]
            if t <= 0:
                continue
            ticks = [0] * N_PROCS
            ticks[p] = t
            di = self.nc.sync.drain()
            wait_clock.add_sem_waits(
                di.ins, ScopedClock({None: VectorClock(ticks)})
            )
        self.nc.all_engine_barrier()
        assert self.sems is not None
        popped = self.nc._tile_sem_poison_stack.pop()
        assert popped is self._sem_poison
        self.nc.clear_and_free_semaphores(list(self.sems.allocated().values()))
        self.nc.all_engine_barrier()

    tile_mod.TileContext._drain_and_barrier = _drain_and_barrier
    tile_mod.TileContext._drain_split_patched = True


def _build():
    import concourse.bass as bass
    import concourse.mybir as mybir
    import concourse.tile as tile

    _patch_drain_split()

    f32 = mybir.dt.float32
    bf16 = mybir.dt.bfloat16

    nc = bass.Bass()
    xe_d = nc.dram_tensor("xe", [P, XE_COLS], f32, kind="ExternalInput")
    wT_d = nc.dram_tensor("wT", [E, IN, OUTP], bf16, kind="ExternalInput")
    y_d = nc.dram_tensor("y", [BL, OUT], f32, kind="ExternalOutput")

    with tile.TileContext(nc) as tc:
        with (
            tc.tile_pool(name="const", bufs=1) as const,
            tc.tile_pool(name="wpool", bufs=1) as wpool,
            tc.tile_pool(name="psum", bufs=2, space="PSUM") as psum,
        ):
            xe = const.tile([P, XE_COLS], f32)
            xs = const.tile([P, E * NIT * BL], bf16)
            y_sb = const.tile([BL, OUT], f32)
            wts = [
                wpool.tile([P, W_DMA_ITILES * OUTP], bf16,
                           name=f"wt{t}", tag=f"wt{t}")
                for t in range(NTILES)
            ]

            nc.sync.dma_start(xe[:], xe_d[:])

            # xs_k[i, b] = xT[i, b] * ew[b, k], downcast to bf16
            for k in range(E):
                for ib in range(NIT):
                    nc.vector.tensor_tensor(
                        xs[:, (k * NIT + ib) * BL:(k * NIT + ib + 1) * BL],
                        xe[:, XT_C + ib * BL:XT_C + (ib + 1) * BL],
                        xe[:, EWB_C + k * BL:EWB_C + (k + 1) * BL],
                        mybir.AluOpType.mult,
                    )

            ps0 = psum.tile([BL, 512], f32)
            ps1 = psum.tile([BL, 512], f32)
            ewt_ap = xe[0:E, EWT_C:EWT_C + BL]
            # bias term: y += ewT.T @ bias (K=8, fp32 matmul - only 2 of them)
            nc.tensor.matmul(
                ps0[:], ewt_ap, xe[0:E, BIAS_C:BIAS_C + 512],
                start=True, stop=False,
            )
            nc.tensor.matmul(
                ps1[:], ewt_ap, xe[0:E, BIAS_C + 512:BIAS_C + 1024],
                start=True, stop=False,
            )

            # wT viewed as a flat stream of E*NIT [128, OUTP] i-blocks,
            # grouped W_DMA_ITILES per DMA/tile.
            wT_flat = wT_d[:].rearrange("k (n p) o -> (k n) p o", p=P)
            for t in range(NTILES):
                wt = wts[t]
                src = wT_flat[t * W_DMA_ITILES:(t + 1) * W_DMA_ITILES].rearrange(
                    "n p o -> p n o"
                )
                dst = wt[:].rearrange("p (n o) -> p n o", n=W_DMA_ITILES)
                nc.sync.dma_start(dst, src)
                # zero matmul: wt-slice.T @ zero-column adds 0 to ps0 but
                # absorbs this tile's DMA-lane wait on its own PE
                # instruction (one-sync-wait walrus limit); its ready-set
                # is a subset of the real matmuls' and its priority is
                # earlier, so it schedules first.
                nc.tensor.matmul(
                    ps0[:, 0:1],
                    wt[:, 0:BL],
                    wt[:, OUT:OUT + 1],
                    start=False, stop=False,
                )
                for j in range(W_DMA_ITILES):
                    blk = t * W_DMA_ITILES + j   # global i-block = k*NIT+ib
                    lhsT = xs[:, blk * BL:(blk + 1) * BL]
                    last = blk == E * NIT - 1
                    nc.tensor.matmul(
                        ps0[:], lhsT,
                        wt[:, j * OUTP:j * OUTP + 512],
                        start=False, stop=last,
                    )
                    nc.tensor.matmul(
                        ps1[:], lhsT,
                        wt[:, j * OUTP + 512:j * OUTP + 1024],
                        start=False, stop=last,
                    )

            nc.vector.tensor_copy(y_sb[:, 0:512], ps0[:])
            nc.vector.tensor_copy(y_sb[:, 512:1024], ps1[:])
            nc.sync.dma_start(y_d[:], y_sb[:])

    return nc


def _get_compiled():
    global _compiled
    if _compiled is None:
        _compiled = _build()
    return _compiled


_wT_cache = None


def _make_in_maps(x, expert_weights, weight, bias):
    global _wT_cache
    import ml_dtypes

    if _wT_cache is None or _wT_cache[0] is not weight:
        wT = np.zeros((E, IN, OUTP), dtype=ml_dtypes.bfloat16)
        wT[:, :, :OUT] = (
            np.asarray(weight, dtype=np.float32)
            .transpose(0, 2, 1)
            .astype(ml_dtypes.bfloat16)
        )
        _wT_cache = (weight, wT)
    wT = _wT_cache[1]
    bias = np.ascontiguousarray(np.asarray(bias, dtype=np.float32))
    x = np.asarray(x, dtype=np.float32)
    ew = np.asarray(expert_weights, dtype=np.float32)
    in_maps = []
    for c in range(NCORES):
        xl = x[c * BL:(c + 1) * BL]          # [64, IN]
        ewl = ew[c * BL:(c + 1) * BL]        # [64, E]
        xe = np.zeros((P, XE_COLS), dtype=np.float32)
        xT = xl.T.reshape(NIT, P, BL)        # [8, 128, 64]
        xe[:, XT_C:XT_C + NIT * BL] = xT.transpose(1, 0, 2).reshape(P, NIT * BL)
        ewb = np.broadcast_to(ewl.T[:, None, :], (E, P, BL))  # [8, 128, 64]
        xe[:, EWB_C:EWB_C + E * BL] = ewb.transpose(1, 0, 2).reshape(P, E * BL)
        xe[0:E, EWT_C:EWT_C + BL] = ewl.T
        xe[0:E, BIAS_C:BIAS_C + OUT] = bias
        in_maps.append({"xe": xe, "wT": wT})
    return in_maps


def kernel(x, expert_weights, weight, bias, _trace=False):
    from concourse.bass_utils import run_bass_kernel_spmd

    nc = _get_compiled()
    in_maps = _make_in_maps(x, expert_weights, weight, bias)
    res = run_bass_kernel_spmd(
        nc, in_maps, core_ids=list(range(NCORES)), trace=_trace
    )
    y = np.concatenate([r["y"] for r in res.results], axis=0).astype(np.float32)
    if _trace:
        return y, res
    return y



# revision 6
# speedup vs baseline: 2.0545x; 2.0545x over previous
"""ExpertLinear (dense MoE blend) Trainium2 kernel — expert-parallel.

y[b,o] = sum_k ew[b,k] * (x[b,:] @ W[k,o,:] + bias[k,o])

Sharding: expert-parallel across 8 cores (core k owns expert k). Each core
computes the full partial z_k = ew[:,k] * (x @ W[k].T + bias[k]) for ALL
512 rows; the host unshard step is a pure sum of the 8 partials. Per-core
HBM traffic is ~4.3 MB (W_k 2 MB bf16 + xT 1 MB bf16 + z_k 1 MB bf16 out)
vs 18.6 MB for the data-parallel layout, because each expert's weights are
read exactly once chip-wide.

Device flow per core (all operands packed in ONE bf16 dram tensor):
  - PE: per o-block group g (8 of them), 8 matmuls (lhsT = W chunk
    [128i,128o] stationary, rhs = xT chunk [128i,512b] moving) accumulate
    zT_k group [128o,512b] into PSUM bank g; all 8 banks live at once,
    each evicted as soon as its group closes so only the last eviction is
    exposed.
  - DVE evict: z = (ps + bias_col) * ew_bcast, downcast bf16. The ew
    blend and bias stay on device; the host only sums partials.
  - Walrus accepts ONE sync wait per instruction and tile emits a sem
    wait for EVERY data dep (even same-engine), so: absorber matmuls
    cover each in-DMA's sem on the PE queue (later matmuls on the same
    sem are coverage-deduped), a tiny DVE copy covers the ew/bias load,
    per-group tmp buffers kill the evict WAR chain, and the DMA count is
    capped at 8 (5 in + 3 out) so no DMAHW sem lane is ever reused (a
    reused lane adds a second wait to that DMA).
  - All DMAs ride the single qSPDynamicHW FIFO ring: the 5 in-DMAs
    stream back-to-back at full bandwidth in issue order, sized so
    compute can start after ~0.8 MB; out-DMAs queue behind them and
    never steal in-stream bandwidth. ~20 warmup matmuls run during the
    initial DMA fill to lift the PE HAM clock gate (1.2 -> 2.4 GHz).
"""

import numpy as np

B, E, IN, OUT = 512, 8, 1024, 1024
NCORES = 8
P = 128
NI = IN // P    # 8 i-chunks (contraction)
NG = OUT // P   # 8 o-block groups
NW = 20         # warmup matmuls

# input column layout (bf16, [128, IN_COLS])
XTA_C = 0                     # xT chunks 0-3: [128, 4*512]
WG0_C = XTA_C + 4 * B         # W group 0: [128, 8*128]
XTB_C = WG0_C + NI * P        # xT chunks 4-7: [128, 4*512]
EWB_C = XTB_C + 4 * B         # ew column broadcast: [128, 512]
BIAS_C = EWB_C + B            # biasT: [128, 8]
WGR_C = BIAS_C + NG           # W groups 1-7: [128, 7*8*128]
IN_COLS = WGR_C + (NG - 1) * NI * P

# in-DMA boundaries: [xTa|Wg0], [xTb|ew|bias], [Wg1-2], [Wg3-4], [Wg5-7]
D0_E = XTB_C
D1_E = WGR_C
D2_E = WGR_C + 2 * NI * P
D3_E = WGR_C + 4 * NI * P


def _wcol(g, i):
    base = WG0_C if g == 0 else WGR_C + (g - 1) * NI * P
    return base + i * P


def _xcol(i):
    return (XTA_C if i < 4 else XTB_C - 4 * B) + i * B


_compiled = None


def _patch_drain_split():
    """The walrus build in this container rejects any instruction carrying
    more than one sync wait, including the kernel-tail Drain that
    TileContext emits with one wait per active semaphore. Split it into a
    sequence of single-wait drains (sequencer-FIFO keeps them ordered;
    the set of waits is identical)."""
    import concourse.tile as tile_mod

    if getattr(tile_mod.TileContext, "_drain_split_patched", False):
        return
    from concourse.tile_sem_assignment import N_PROCS
    from concourse.vector_clock import ScopedClock, VectorClock

    def _drain_and_barrier(self, tick_clock, wait_clock):
        gc = tick_clock.global_clock
        for p in range(N_PROCS):
            t = gc[p]
            if t <= 0:
                continue
            ticks = [0] * N_PROCS
            ticks[p] = t
            di = self.nc.sync.drain()
            wait_clock.add_sem_waits(
                di.ins, ScopedClock({None: VectorClock(ticks)})
            )
        self.nc.all_engine_barrier()
        assert self.sems is not None
        popped = self.nc._tile_sem_poison_stack.pop()
        assert popped is self._sem_poison
        self.nc.clear_and_free_semaphores(list(self.sems.allocated().values()))
        self.nc.all_engine_barrier()

    tile_mod.TileContext._drain_and_barrier = _drain_and_barrier
    tile_mod.TileContext._drain_split_patched = True


def _build():
    import concourse.bass as bass
    import concourse.mybir as mybir
    import concourse.tile as tile

    _patch_drain_split()

    f32 = mybir.dt.float32
    bf16 = mybir.dt.bfloat16

    nc = bass.Bass()
    in_d = nc.dram_tensor("inp", [P, IN_COLS], bf16, kind="ExternalInput")
    z_d = nc.dram_tensor("z", [P, NG * B], bf16, kind="ExternalOutput")

    with tile.TileContext(nc) as tc:
        with (
            tc.tile_pool(name="const", bufs=1) as const,
            tc.tile_pool(name="psum", bufs=1, space="PSUM") as psum,
        ):
            inp = const.tile([P, IN_COLS], bf16)
            warm = const.tile([P, P], bf16)
            eb32 = const.tile([P, B + NG], f32)   # ew32 | b32
            dsc = const.tile([1, 8], f32)         # DVE absorber scratch
            tmps = [const.tile([P, B], f32, name=f"tmp{g}", tag=f"tmp{g}")
                    for g in range(NG)]
            zsb = const.tile([P, NG * B], bf16)
            ps = [psum.tile([P, B], f32, name=f"ps{g}", tag=f"ps{g}")
                  for g in range(NG)]

            # --- in-DMAs: 5, issue order = FIFO stream order ---
            for lo, hi in ((0, D0_E), (D0_E, D1_E), (D1_E, D2_E),
                           (D2_E, D3_E), (D3_E, IN_COLS)):
                nc.sync.dma_start(inp[:, lo:hi], in_d[:, lo:hi])

            # --- PE warmup: lift the HAM clock gate while DMAs fill ---
            nc.vector.memset(warm[:], 1.0)
            for w in range(NW):
                nc.tensor.matmul(
                    ps[NG - 1][0:1, 0:P], warm[:, 0:1], warm[:, 0:P],
                    start=True, stop=True, skip_group_check=True,
                )

            # --- DVE: upcast ew|bias, then cover its tick for later ops ---
            nc.vector.tensor_copy(eb32[:], inp[:, EWB_C:EWB_C + B + NG])
            nc.vector.tensor_copy(dsc[0:1, 0:1], eb32[0:1, 0:1])

            # --- PE absorbers: cover the two xT DMA sems ---
            def absorber(c):
                nc.tensor.matmul(
                    ps[NG - 1][0:1, 0:1], inp[:, c:c + 1], inp[:, c + 1:c + 2],
                    start=True, stop=True, skip_group_check=True,
                )

            absorber(XTA_C)        # D0 (also covers Wg0)
            for g in range(NG):
                for i in range(NI):
                    if g == 0 and i == 4:
                        absorber(XTB_C)   # D1 (also covers ew/bias)
                    nc.tensor.matmul(
                        ps[g][:],
                        inp[:, _wcol(g, i):_wcol(g, i) + P],
                        inp[:, _xcol(i):_xcol(i) + B],
                        start=(i == 0), stop=(i == NI - 1),
                    )
                # evict group g as soon as its accumulation closes
                nc.vector.tensor_tensor(
                    tmps[g][:], ps[g][:],
                    eb32[:, B + g:B + g + 1].broadcast_to([P, B]),
                    mybir.AluOpType.add,
                )
                nc.vector.tensor_tensor(
                    zsb[:, g * B:(g + 1) * B], tmps[g][:], eb32[:, 0:B],
                    mybir.AluOpType.mult,
                )
                if g in (2, 5, 7):
                    lo = 0 if g == 2 else (3 * B if g == 5 else 6 * B)
                    hi = (g + 1) * B
                    nc.sync.dma_start(z_d[:, lo:hi], zsb[:, lo:hi])

    return nc


def _get_compiled():
    global _compiled
    if _compiled is None:
        _compiled = _build()
    return _compiled


_prep_cache = None


def _make_in_maps(x, expert_weights, weight, bias):
    global _prep_cache
    import ml_dtypes

    bf = ml_dtypes.bfloat16
    if _prep_cache is None or _prep_cache[0] is not weight:
        wt = np.asarray(weight, dtype=np.float32)
        # wg[p, (g,i,c)] = W[k, g*128+c, i*128+p]
        wgs = [
            wt[k].T.reshape(NI, P, NG, P)
            .transpose(1, 2, 0, 3).reshape(P, NG * NI * P).astype(bf)
            for k in range(NCORES)
        ]
        _prep_cache = (weight, wgs)
    wgs = _prep_cache[1]
    # xT chunks: xt[p, i*512+b] = x[b, i*128+p] — same bytes every core
    xt = (np.asarray(x, dtype=np.float32).T.reshape(NI, P, B)
          .transpose(1, 0, 2).reshape(P, NI * B).astype(bf))
    ew = np.asarray(expert_weights, dtype=np.float32)
    bs = np.asarray(bias, dtype=np.float32)
    in_maps = []
    for k in range(NCORES):
        inp = np.empty((P, IN_COLS), dtype=bf)
        inp[:, XTA_C:XTA_C + 4 * B] = xt[:, :4 * B]
        inp[:, XTB_C:XTB_C + 4 * B] = xt[:, 4 * B:]
        inp[:, WG0_C:WG0_C + NI * P] = wgs[k][:, :NI * P]
        inp[:, WGR_C:IN_COLS] = wgs[k][:, NI * P:]
        inp[:, EWB_C:EWB_C + B] = np.broadcast_to(ew[:, k], (P, B))
        inp[:, BIAS_C:BIAS_C + NG] = bs[k].reshape(NG, P).T
        in_maps.append({"inp": inp})
    return in_maps


def kernel(x, expert_weights, weight, bias, _trace=False):
    from concourse.bass_utils import run_bass_kernel_spmd

    nc = _get_compiled()
    in_maps = _make_in_maps(x, expert_weights, weight, bias)
    res = run_bass_kernel_spmd(
        nc, in_maps, core_ids=list(range(NCORES)), trace=_trace
    )
    y = np.zeros((B, OUT), dtype=np.float32)
    for r in res.results:
        z = np.asarray(r["z"]).astype(np.float32)  # [128, 8*512]
        y += z.reshape(P, NG, B).transpose(1, 0, 2).reshape(OUT, B).T
    if _trace:
        return y, res
    return y


# revision 8
# speedup vs baseline: 2.1746x; 1.0585x over previous
"""ExpertLinear (dense MoE blend) Trainium2 kernel — expert-parallel.

y[b,o] = sum_k ew[b,k] * (x[b,:] @ W[k,o,:] + bias[k,o])

Sharding: expert-parallel across 8 cores (core k owns expert k). Each core
computes the full partial z_k = ew[:,k] * (x @ W[k].T + bias[k]) for ALL
512 rows; the host unshard step is a pure sum of the 8 partials. Per-core
HBM traffic is ~4.3 MB (W_k 2 MB bf16 + xT 1 MB bf16 + z_k 1 MB bf16 out)
vs 18.6 MB for the data-parallel layout, because each expert's weights are
read exactly once chip-wide.

Device flow per core (all operands packed in ONE bf16 dram tensor):
  - PE: per o-block group g (8 of them), 8 matmuls (lhsT = W chunk
    [128i,128o] stationary, rhs = xT chunk [128i,512b] moving) accumulate
    zT_k group [128o,512b] into PSUM bank g; all 8 banks live at once,
    each evicted as soon as its group closes so only the last eviction is
    exposed.
  - DVE evict: z = (ps + bias_col) * ew_bcast, downcast bf16. The ew
    blend and bias stay on device; the host only sums partials.
  - Walrus accepts ONE sync wait per instruction and tile emits a sem
    wait for EVERY data dep (even same-engine), so: absorber matmuls
    cover each in-DMA's sem on the PE queue (later matmuls on the same
    sem are coverage-deduped), a tiny DVE copy covers the ew/bias load,
    per-group tmp buffers kill the evict WAR chain, and the DMA count is
    capped at 8 (5 in + 3 out) so no DMAHW sem lane is ever reused (a
    reused lane adds a second wait to that DMA).
  - All DMAs ride the single qSPDynamicHW FIFO ring: the 5 in-DMAs
    stream back-to-back at full bandwidth in issue order, sized so
    compute can start after ~0.8 MB; out-DMAs queue behind them and
    never steal in-stream bandwidth. ~20 warmup matmuls run during the
    initial DMA fill to lift the PE HAM clock gate (1.2 -> 2.4 GHz).
"""

import numpy as np

B, E, IN, OUT = 512, 8, 1024, 1024
NCORES = 8
P = 128
NI = IN // P    # 8 i-chunks (contraction)
NG = OUT // P   # 8 o-block groups
NW = 40         # warmup matmuls (bridge PE from preamble exit to first W arrival)

# input column layout (bf16, [128, IN_COLS])
XTA_C = 0                     # xT chunks 0-3: [128, 4*512]
WG0_C = XTA_C + 4 * B         # W group 0: [128, 8*128]
XTB_C = WG0_C + NI * P        # xT chunks 4-7: [128, 4*512]
EWB_C = XTB_C + 4 * B         # ew column broadcast: [128, 512]
BIAS_C = EWB_C + B            # biasT: [128, 8]
WGR_C = BIAS_C + NG           # W groups 1-7: [128, 7*8*128]
IN_COLS = WGR_C + (NG - 1) * NI * P

# in-DMA boundaries: [xTa|Wg0], [xTb|ew|bias], [Wg1-2], [Wg3-4], [Wg5-7]
D0_E = XTB_C
D1_E = WGR_C
D2_E = WGR_C + 2 * NI * P
D3_E = WGR_C + 4 * NI * P


def _wcol(g, i):
    base = WG0_C if g == 0 else WGR_C + (g - 1) * NI * P
    return base + i * P


def _xcol(i):
    return (XTA_C if i < 4 else XTB_C - 4 * B) + i * B


_compiled = None


def _patch_drain_split():
    """The walrus build in this container rejects any instruction carrying
    more than one sync wait, including the kernel-tail Drain that
    TileContext emits with one wait per active semaphore. Split it into a
    sequence of single-wait drains (sequencer-FIFO keeps them ordered;
    the set of waits is identical)."""
    import concourse.tile as tile_mod

    if getattr(tile_mod.TileContext, "_drain_split_patched", False):
        return
    from concourse.tile_sem_assignment import N_PROCS
    from concourse.vector_clock import ScopedClock, VectorClock

    def _drain_and_barrier(self, tick_clock, wait_clock):
        gc = tick_clock.global_clock
        for p in range(N_PROCS):
            t = gc[p]
            if t <= 0:
                continue
            ticks = [0] * N_PROCS
            ticks[p] = t
            di = self.nc.sync.drain()
            wait_clock.add_sem_waits(
                di.ins, ScopedClock({None: VectorClock(ticks)})
            )
        self.nc.all_engine_barrier()
        assert self.sems is not None
        popped = self.nc._tile_sem_poison_stack.pop()
        assert popped is self._sem_poison
        self.nc.clear_and_free_semaphores(list(self.sems.allocated().values()))
        self.nc.all_engine_barrier()

    tile_mod.TileContext._drain_and_barrier = _drain_and_barrier
    tile_mod.TileContext._drain_split_patched = True


def _build():
    import concourse.bass as bass
    import concourse.mybir as mybir
    import concourse.tile as tile

    _patch_drain_split()

    f32 = mybir.dt.float32
    bf16 = mybir.dt.bfloat16

    nc = bass.Bass()
    in_d = nc.dram_tensor("inp", [P, IN_COLS], bf16, kind="ExternalInput")
    z_d = nc.dram_tensor("z", [P, NG * B], bf16, kind="ExternalOutput")

    with tile.TileContext(nc) as tc:
        with (
            tc.tile_pool(name="const", bufs=1) as const,
            tc.tile_pool(name="psum", bufs=1, space="PSUM") as psum,
        ):
            inp = const.tile([P, IN_COLS], bf16)
            warm = const.tile([P, P], bf16)
            eb32 = const.tile([P, B + NG], f32)   # ew32 | b32
            dsc = const.tile([1, 8], f32)         # DVE absorber scratch
            tmps = [const.tile([P, B], f32, name=f"tmp{g}", tag=f"tmp{g}")
                    for g in range(NG)]
            zsb = const.tile([P, NG * B], bf16)
            ps = [psum.tile([P, B], f32, name=f"ps{g}", tag=f"ps{g}")
                  for g in range(NG)]

            # --- in-DMAs: 5, issue order = FIFO stream order ---
            for lo, hi in ((0, D0_E), (D0_E, D1_E), (D1_E, D2_E),
                           (D2_E, D3_E), (D3_E, IN_COLS)):
                nc.sync.dma_start(inp[:, lo:hi], in_d[:, lo:hi])

            # --- PE warmup: lift the HAM clock gate while DMAs fill ---
            nc.vector.memset(warm[:], 1.0)
            for w in range(NW):
                nc.tensor.matmul(
                    ps[NG - 1][0:1, 0:P], warm[:, 0:1], warm[:, 0:P],
                    start=True, stop=True, skip_group_check=True,
                )

            # --- DVE: upcast ew|bias, then cover its tick for later ops ---
            nc.vector.tensor_copy(eb32[:], inp[:, EWB_C:EWB_C + B + NG])
            nc.vector.tensor_copy(dsc[0:1, 0:1], eb32[0:1, 0:1])

            # --- PE absorbers: cover the two xT DMA sems ---
            def absorber(c):
                nc.tensor.matmul(
                    ps[NG - 1][0:1, 0:1], inp[:, c:c + 1], inp[:, c + 1:c + 2],
                    start=True, stop=True, skip_group_check=True,
                )

            absorber(XTA_C)        # D0 (also covers Wg0)
            for g in range(NG):
                for i in range(NI):
                    if g == 0 and i == 4:
                        absorber(XTB_C)   # D1 (also covers ew/bias)
                    nc.tensor.matmul(
                        ps[g][:],
                        inp[:, _wcol(g, i):_wcol(g, i) + P],
                        inp[:, _xcol(i):_xcol(i) + B],
                        start=(i == 0), stop=(i == NI - 1),
                    )
                # evict group g as soon as its accumulation closes
                nc.vector.tensor_tensor(
                    tmps[g][:], ps[g][:],
                    eb32[:, B + g:B + g + 1].broadcast_to([P, B]),
                    mybir.AluOpType.add,
                )
                nc.vector.tensor_tensor(
                    zsb[:, g * B:(g + 1) * B], tmps[g][:], eb32[:, 0:B],
                    mybir.AluOpType.mult,
                )
                if g in (3, 6, 7):
                    lo = 0 if g == 3 else (4 * B if g == 6 else 7 * B)
                    hi = (g + 1) * B
                    nc.sync.dma_start(z_d[:, lo:hi], zsb[:, lo:hi])

    return nc


def _get_compiled():
    global _compiled
    if _compiled is None:
        _compiled = _build()
    return _compiled


_prep_cache = None


def _make_in_maps(x, expert_weights, weight, bias):
    global _prep_cache
    import ml_dtypes

    bf = ml_dtypes.bfloat16
    if _prep_cache is None or _prep_cache[0] is not weight:
        wt = np.asarray(weight, dtype=np.float32)
        # wg[p, (g,i,c)] = W[k, g*128+c, i*128+p]
        wgs = [
            wt[k].T.reshape(NI, P, NG, P)
            .transpose(1, 2, 0, 3).reshape(P, NG * NI * P).astype(bf)
            for k in range(NCORES)
        ]
        _prep_cache = (weight, wgs)
    wgs = _prep_cache[1]
    # xT chunks: xt[p, i*512+b] = x[b, i*128+p] — same bytes every core
    xt = (np.asarray(x, dtype=np.float32).T.reshape(NI, P, B)
          .transpose(1, 0, 2).reshape(P, NI * B).astype(bf))
    ew = np.asarray(expert_weights, dtype=np.float32)
    bs = np.asarray(bias, dtype=np.float32)
    in_maps = []
    for k in range(NCORES):
        inp = np.empty((P, IN_COLS), dtype=bf)
        inp[:, XTA_C:XTA_C + 4 * B] = xt[:, :4 * B]
        inp[:, XTB_C:XTB_C + 4 * B] = xt[:, 4 * B:]
        inp[:, WG0_C:WG0_C + NI * P] = wgs[k][:, :NI * P]
        inp[:, WGR_C:IN_COLS] = wgs[k][:, NI * P:]
        inp[:, EWB_C:EWB_C + B] = np.broadcast_to(ew[:, k], (P, B))
        inp[:, BIAS_C:BIAS_C + NG] = bs[k].reshape(NG, P).T
        in_maps.append({"inp": inp})
    return in_maps


def kernel(x, expert_weights, weight, bias, _trace=False):
    from concourse.bass_utils import run_bass_kernel_spmd

    nc = _get_compiled()
    in_maps = _make_in_maps(x, expert_weights, weight, bias)
    res = run_bass_kernel_spmd(
        nc, in_maps, core_ids=list(range(NCORES)), trace=_trace
    )
    y = np.zeros((B, OUT), dtype=np.float32)
    for r in res.results:
        z = np.asarray(r["z"]).astype(np.float32)  # [128, 8*512]
        y += z.reshape(P, NG, B).transpose(1, 0, 2).reshape(OUT, B).T
    if _trace:
        return y, res
    return y
